# revision 74
# baseline (speedup 1.0000x reference)
"""Multi-head attention (B=2, S=2048, D=1024, H=16) on 8 TRN2 NeuronCores.

Sharding: batch x head-group. Core c handles batch c//4 and heads
[4*(c%4), 4*(c%4)+4). Each core computes its heads' Q/K/V projections
(column-parallel), causal attention, and a row-parallel partial of the
output projection. The host sums the 4 partials per batch and adds dense_b.

All matmul operands are fp16 (fp32 PSUM accumulation); matmul cost is
streamed-rows-bound, so every matmul carries a full 128x128 payload:
  QT/KT [128, 2, S] fp16: partition = head-pair-local feature (2 heads x 64),
    second dim = head pair (pc). V [128p=seq, chunk, head, 65] fp16 with a
    ones column (col 64) that accumulates the softmax denominator.
  logits (phase LB): L[:, i, off:] = KT_chunk.T @ QT_block (2 heads
    row-packed via tile_position, columns below the causal diagonal
    skipped); PT = exp(0.125*L) on ScalarE (a quarter of the full chunks
    use a Schraudolph bit-trick exp on DVE instead -- ScalarE is the
    phase-B bottleneck); diagonal 128-block masked multiplicatively with
    a 0/1 lower-tri pattern on the Pool engine.
  PV (phase PB): natural orientation per head i, q-sub qs (128 wide,
    qs >= chunk diagonal): O[i][:, qs, 0:65] += PT[:, i, qs-cols].T @ V_aug
    -- K=128, M=128, only 65 streamed rows per chunk. Each O bank hosts
    strictly sequential accumulation groups (interleaved groups within one
    PSUM bank corrupt each other).
  normalize: recip = 1/O[:, :, 64] (DVE reciprocal); O8 = O * recip fused
    into the PSUM evac; OT = PE-transpose(O8) per q-sub into a shared PSUM
    tile; one DVE copy to OT_sb [128, 2, 4, 128] fp16.
  dense: outT[mc*128:+128, q-block] = sum_t dnT[:, t, mc].T @ OT_sb[:, t],
    evacuated on DVE (Act for the drain tail) and DMA'd out in 4-wide
    batches.

Emission is unit-interleaved so the in-order engines stay fed:
A0 | A1(x)LB0 | A2(x)LB1 | A3(x)LB2 | PB0(x)LB3 | PB1 | PB2 | PB3
(phase A is PE-bound while Act idles; LB is Act-bound while PE idles;
PB is PE-heavy). PT tiles are double-buffered per (pc, kc) tag so an
LB(j+1) write WARs against PB(j-1)'s long-done reads instead of
serializing the pipeline on PB(j)'s future reads.
"""

import numpy as np
from contextlib import ExitStack

import concourse.tile as tile
from concourse import bacc, mybir
from concourse.bass_utils import run_bass_kernel_spmd

F32 = mybir.dt.float32
F16 = mybir.dt.float16
I32 = mybir.dt.int32
AF = mybir.ActivationFunctionType
ADD = mybir.AluOpType.add
MULT = mybir.AluOpType.mult

B, S, D, H = 2, 2048, 1024, 16
NCORES = 8
HL = 4            # heads per core
DH = D // H       # 64
DLOC = HL * DH    # 256 local feature dims
SBK = 512         # seq block (q)
NSB = S // SBK    # 4
KCH = 128         # k chunk
NCH = S // KCH    # 16
# Schraudolph exp constants (0.125 softmax scale folded in): exp(0.125*x)
# ~= bitcast_f32(int32(x*EA + EB)); ~3% max relative error
EA = float(2 ** 23 / np.log(2) * 0.125)
EB = float(127 * 2 ** 23 - 0.043677448 * 2 ** 23)


def _ts(i, n):
    return slice(i * n, (i + 1) * n)


def build(debug=False):
    nc = bacc.Bacc(None, target_bir_lowering=False)

    xqT = nc.dram_tensor("xqT", [D, S], F16, kind="ExternalInput")
    xkT = nc.dram_tensor("xkT", [D, S], F16, kind="ExternalInput")
    xvT = nc.dram_tensor("xvT", [D, S], F16, kind="ExternalInput")
    wqT = nc.dram_tensor("wqT", [D, DLOC], F16, kind="ExternalInput")
    wkT = nc.dram_tensor("wkT", [D, DLOC], F16, kind="ExternalInput")
    wvT = nc.dram_tensor("wvT", [D, DLOC], F16, kind="ExternalInput")
    dnT = nc.dram_tensor("dnT", [DLOC, D], F16, kind="ExternalInput")
    outT = nc.dram_tensor("outT", [D, S], F16, kind="ExternalOutput")
    if debug:
        dQT = nc.dram_tensor("dQT", [128, 2, S], F16, kind="ExternalOutput")
        dKT = nc.dram_tensor("dKT", [128, 2, S], F16, kind="ExternalOutput")
        dV = nc.dram_tensor("dV", [128, NCH, HL, DH + 1], F16, kind="ExternalOutput")
        dPT = nc.dram_tensor("dPT", [128, 2, SBK], F16, kind="ExternalOutput")
        dO = nc.dram_tensor("dO", [2, 128, NSB, 128], F32, kind="ExternalOutput")
        dO8 = nc.dram_tensor("dO8", [2, 128, NSB, DH], F16, kind="ExternalOutput")
        dOT = nc.dram_tensor("dOT", [128, 2, NSB, 128], F16, kind="ExternalOutput")

    # lower-tri 0/1 pattern (allowed = k <= q within the diagonal block)
    tri_np = (np.arange(128)[:, None] <= np.arange(128)[None, :]).astype(np.float16)
    tri_c = nc.inline_tensor(tri_np, name="tri01")
    id_c = nc.inline_tensor(np.eye(128, dtype=np.float16), name="id128")

    with tile.TileContext(nc) as tc, ExitStack() as ctx:
        pers = ctx.enter_context(tc.tile_pool(name="pers", bufs=1))
        xpool = ctx.enter_context(tc.tile_pool(name="xpool", bufs=14))
        # PT pools: double-buffered tags so LB(j+1) writes WAR against
        # PB(j-1)'s (long-done) reads, not PB(j)'s future reads; kc>=12 tags
        # are only used by j=3 (single version) so they stay single-buffered
        ptpA = ctx.enter_context(tc.tile_pool(name="ptpA", bufs=2))
        ptpB = ctx.enter_context(tc.tile_pool(name="ptpB", bufs=1))
        o8p = ctx.enter_context(tc.tile_pool(name="o8p", bufs=1))
        otp = ctx.enter_context(tc.tile_pool(name="otp", bufs=2))
        evp = ctx.enter_context(tc.tile_pool(name="evp", bufs=3))
        smallp = ctx.enter_context(tc.tile_pool(name="smallp", bufs=4))
        schp = ctx.enter_context(tc.tile_pool(name="schp", bufs=2))
        mmp = ctx.enter_context(tc.tile_pool(name="mmp", bufs=2, space="PSUM"))
        lp = ctx.enter_context(tc.tile_pool(name="lp", bufs=2, space="PSUM"))
        onp = ctx.enter_context(tc.tile_pool(name="onp", bufs=1, space="PSUM"))  # 2 tags x 1 buf

        # ---------- persistent tiles ----------
        wparts = {}
        for wname in ("q", "k", "v"):
            wparts[wname] = pers.tile([128, 8, DLOC], F16, tag=f"w{wname}",
                                      name=f"w_{wname}")
        dn_sb = pers.tile([128, 2, D], F16, tag="dn")
        tri_sb = pers.tile([128, 128], F16, tag="tri")
        id_sb = pers.tile([128, 128], F16, tag="id")

        QT_sb = pers.tile([128, 2, S], F16, tag="QT")
        KT_sb = pers.tile([128, 2, S], F16, tag="KT")
        V_sb = pers.tile([128, NCH, HL, DH + 1], F16, tag="V")
        # ones column (softmax denominator accumulates via PV matmul)
        nc.vector.memset(V_sb[:, :, :, DH:DH + 1], 1.0)

        outT_r = outT.rearrange("(c p) s -> p c s", p=128)

        def load_one(xname, j, js, fine=False):
            # returns one AP of shape [128, 512] per contraction chunk kc
            src = {"q": xqT, "k": xkT, "v": xvT}[xname]
            srcr = src.rearrange("(c p) s -> p c s", p=128)
            aps = []
            for i in range(4):
                t = xpool.tile([128, 2, SBK], F16, tag="xt",
                               name=f"x_{xname}_{j}_{i}")
                nc.sync.dma_start(out=t, in_=srcr[:, _ts(i, 2), js])
                aps.extend(t[:, c, :] for c in range(2))
            return aps

        def load_x(j, js):
            return {xname: load_one(xname, j, js) for xname in ("q", "k", "v")}

        def a_units(j, js, xt, split_v=False):
            # projection series closures; evacs on Act (idle during phase A)
            units = []

            def qk_unit(bname, dst, mc):
                def f():
                    ps = mmp.tile([128, 512], F32, tag="mm",
                                  name=f"ps_{bname}_{j}_{mc}")
                    for kc in range(8):
                        nc.tensor.matmul(
                            ps[:, :],
                            lhsT=wparts[bname][:, kc, _ts(mc, 128)],
                            rhs=xt[bname][kc],
                            start=(kc == 0), stop=(kc == 7),
                        )
                    nc.scalar.copy(dst[:, mc, js], ps)
                return f

            def v_unit(sc):
                def f():
                    ps = mmp.tile([128, 512], F32, tag="mm",
                                  name=f"ps_v_{j}_{sc}")
                    for kc in range(8):
                        nc.tensor.matmul(
                            ps[:, 0:DLOC],
                            lhsT=xt["v"][kc][:, _ts(sc, 128)],
                            rhs=wparts["v"][:, kc, :],
                            start=(kc == 0), stop=(kc == 7),
                        )
                    nc.scalar.copy(
                        V_sb[:, j * 4 + sc, :, 0:DH],
                        ps[:, 0:DLOC].rearrange("p (h d) -> p h d", h=HL),
                    )
                return f

            for bname, dst in (("q", QT_sb), ("k", KT_sb)):
                for mc in range(2):
                    units.append(qk_unit(bname, dst, mc))
            vu = [v_unit(sc) for sc in range(4)]
            if split_v:
                return units, vu
            return units + vu

        def lb_units(j, js, PTs):
            # per-(pc, kc) logits+exp closures; PTs[pc][kc] filled at emission
            nkc = (j + 1) * 4

            def unit(pc, kc):
                def f():
                    off = max(0, kc - 4 * j) * KCH  # causal column trim
                    L = lp.tile([128, 2, SBK], F32, tag="L",
                                name=f"L_{j}_{pc}_{kc}")
                    for i in range(2):
                        nc.tensor.matmul(
                            L[:, i, off:SBK],
                            lhsT=KT_sb[_ts(i, 64), pc, _ts(kc, KCH)],
                            rhs=QT_sb[_ts(i, 64), pc, j * SBK + off:(j + 1) * SBK],
                            start=True, stop=True,
                            tile_position=(64 * i, 0),
                        )
                    pool = ptpA if kc < 12 else ptpB
                    PT = pool.tile([128, 2, SBK], F16, tag=f"PT{pc}_{kc}",
                                   name=f"PT_{j}_{pc}_{kc}")
                    if kc < 4 * j and (kc % 4 == 1 or (j == 3 and kc % 4 == 3)):
                        # offload this full chunk's exp to DVE (Schraudolph
                        # bit-trick): Act is the phase-B bottleneck engine
                        T = schp.tile([128, 2, SBK], I32, tag="sch",
                                      name=f"T_{j}_{pc}_{kc}")
                        nc.vector.tensor_scalar(
                            out=T, in0=L, scalar1=EA, scalar2=EB,
                            op0=MULT, op1=ADD)
                        nc.gpsimd.tensor_copy(PT, T.bitcast(F32))
                    else:
                        nc.scalar.activation(
                            out=PT[:, :, off:SBK], in_=L[:, :, off:SBK],
                            func=AF.Exp, scale=0.125)
                    if kc >= 4 * j:
                        # mask the diagonal 128-block (0/1 lower-tri multiply)
                        # on the otherwise-idle Pool engine (SBUF-only op)
                        nc.gpsimd.tensor_tensor(
                            out=PT[:, :, off:off + KCH],
                            in0=PT[:, :, off:off + KCH],
                            in1=tri_sb[:, None, :].broadcast_to([128, 2, KCH]),
                            op=MULT,
                        )
                    if debug and j == 0 and pc == 0 and kc == 0:
                        nc.sync.dma_start(out=dPT[:, :, :], in_=PT)
                    PTs[pc].append(PT)
                return f

            return [unit(pc, kc) for pc in range(2) for kc in range(nkc)]

        def pb_units(j, js, PTs):
            # PV + normalize + transpose + dense closures (PE-heavy)
            state = {}
            Os, O8s = {}, {}

            def emit_ot():
                state["OT"] = otp.tile([128, 2, NSB, 128], F16, tag="ot",
                                       name=f"OT_{j}")

            def emit_pv(pc, i):
                O = onp.tile([128, NSB, 128], F32, tag=f"o{i}",
                             name=f"O_{j}_{pc}_{i}")
                for qs in range(NSB):
                    kmax = 4 * j + qs
                    for kc in range(kmax + 1):
                        nc.tensor.matmul(
                            O[:, qs, 0:DH + 1],
                            lhsT=PTs[pc][kc][:, i, _ts(qs, 128)],
                            rhs=V_sb[:, kc, 2 * pc + i, :],
                            start=(kc == 0), stop=(kc == kmax),
                            skip_group_check=True,
                        )
                Os[pc, i] = O

            def emit_norm(pc, i):
                O = Os[pc, i]
                rc = smallp.tile([128, NSB, 1], F32, tag="rc")
                nc.vector.reciprocal(rc, O[:, :, DH:DH + 1])
                if i == 0:
                    # both heads share one tile: [128q, qs, (i, d)] so the
                    # transpose below covers the head pair in one pass
                    O8s[pc] = o8p.tile([128, NSB, 2, DH], F16, tag=f"o8{pc}",
                                       name=f"O8_{j}_{pc}")
                nc.vector.tensor_tensor(
                    out=O8s[pc][:, :, i, :], in0=O[:, :, 0:DH],
                    in1=rc.broadcast_to([128, NSB, DH]), op=MULT,
                )
                if debug and j == 0 and pc == 0:
                    ostage = evp.tile([128, NSB, 128], F32, tag="ev",
                                      name=f"ostage_{i}")
                    nc.vector.tensor_copy(ostage, O)
                    nc.sync.dma_start(out=dO[i, :, :, :], in_=ostage)
                    nc.sync.dma_start(out=dO8[i, :, :, :], in_=O8s[pc][:, :, i, :])

            def emit_transposes(pc, pool=None, tag=None):
                # one 128-wide transpose per q-sub covers both heads: out
                # partition f = i*64 + d, exactly OT's layout
                tp = (pool or mmp).tile([128, NSB, 128], F16, tag=(tag or "mm"),
                                        name=f"tp_{j}_{pc}")
                for qs in range(NSB):
                    nc.tensor.transpose(
                        tp[:, qs, :], O8s[pc][:, qs, :, :], id_sb)
                nc.vector.tensor_copy(state["OT"][:, pc, :, :], tp)

            def dense_unit(mc):
                def f():
                    OT = state["OT"]
                    last = j == NSB - 1
                    # the last block may borrow the (now idle) logits PSUM
                    # pool for a deeper dense ring
                    pool = lp if (last and mc % 2 == 1) else mmp
                    dps = pool.tile([128, 512], F32,
                                    tag=("L" if pool is lp else "mm"),
                                    name=f"dps_{j}_{mc}")
                    for t in range(2):
                        nc.tensor.matmul(
                            dps[:, :],
                            lhsT=dn_sb[:, t, _ts(mc, 128)],
                            rhs=OT[:, t, :, :].rearrange("p a b -> p (a b)"),
                            start=(t == 0), stop=(t == 1),
                        )
                    # 4-wide staging tile; one DMA per 4 mc's (fewer DMA
                    # dispatch/sem overheads); the last block's final batch
                    # goes out as two 2-wide DMAs
                    if mc % 4 == 0:
                        state["ev"] = evp.tile([128, 4, 512], F16, tag="ev",
                                               name=f"ev_{j}_{mc // 4}")
                    ev = state["ev"]
                    if last and mc >= 6:
                        nc.vector.tensor_copy(ev[:, mc % 4, 0:256], dps[:, 0:256])
                        nc.scalar.copy(ev[:, mc % 4, 256:512], dps[:, 256:512])
                    elif last and mc % 2 == 1:
                        nc.scalar.copy(ev[:, mc % 4, :], dps)
                    else:
                        nc.vector.tensor_copy(ev[:, mc % 4, :], dps)
                    if mc % 4 == 3:
                        if last and mc == 7:
                            nc.sync.dma_start(
                                out=outT_r[:, 4:6, js], in_=ev[:, 0:2, :])
                            nc.sync.dma_start(
                                out=outT_r[:, 6:8, js], in_=ev[:, 2:4, :])
                        else:
                            nc.sync.dma_start(
                                out=outT_r[:, mc - 3:mc + 1, js], in_=ev)
                return f

            def dbg_ot():
                if debug and j == 0:
                    nc.sync.dma_start(out=dOT[:, :, :, :], in_=state["OT"])

            def dense_t0_unit(mc):
                # open a dense accumulation with only the pc0 contribution
                # (emittable as soon as OT[:, 0] exists, during pv(1, 1))
                def f():
                    OT = state["OT"]
                    pool = lp if mc >= 2 else mmp
                    dps = pool.tile([128, 512], F32,
                                    tag=("L" if pool is lp else "mm"),
                                    name=f"dps_{j}_{mc}")
                    state[f"dps{mc}"] = dps
                    nc.tensor.matmul(
                        dps[:, :], lhsT=dn_sb[:, 0, _ts(mc, 128)],
                        rhs=OT[:, 0, :, :].rearrange("p a b -> p (a b)"),
                        start=True, stop=False, skip_group_check=True,
                    )
                return f

            def dense_t1_unit(mc):
                def f():
                    OT = state["OT"]
                    dps = state[f"dps{mc}"]
                    nc.tensor.matmul(
                        dps[:, :], lhsT=dn_sb[:, 1, _ts(mc, 128)],
                        rhs=OT[:, 1, :, :].rearrange("p a b -> p (a b)"),
                        start=False, stop=True, skip_group_check=True,
                    )
                    if mc % 2 == 0:
                        state["ev"] = evp.tile([128, 2, 512], F16, tag="ev",
                                               name=f"ev_{j}_{mc // 2}")
                    ev = state["ev"]
                    if mc % 2 == 1:
                        nc.scalar.copy(ev[:, 1, :], dps)
                        nc.sync.dma_start(
                            out=outT_r[:, mc - 1:mc + 1, js], in_=ev)
                    else:
                        nc.vector.tensor_copy(ev[:, 0, :], dps)
                return f

            units = [emit_ot]
            units.append(lambda: emit_pv(0, 0))
            units.append(lambda: emit_norm(0, 0))
            units.append(lambda: emit_pv(0, 1))
            units.append(lambda: emit_norm(0, 1))
            units.append(lambda: emit_pv(1, 0))
            units.append(lambda: emit_norm(1, 0))
            units.append(lambda: emit_transposes(0))
            if False:  # endgame split: no measured gain over simple path
                # endgame: pv(1,1)'s PE time covers norm(1,1); the open pc0
                # dense halves cover the pc1 transpose/OT-evac chain
                units.append(lambda: emit_pv(1, 1))
                units.append(lambda: emit_norm(1, 1))
                for mc in range(4):
                    units.append(dense_t0_unit(mc))
                units.append(lambda: emit_transposes(1, pool=onp, tag="o0"))
                units.append(dbg_ot)
                for mc in range(4):
                    units.append(dense_t1_unit(mc))
                for mc in range(4, 8):
                    units.append(dense_unit(mc))
            else:
                units.append(lambda: emit_pv(1, 1))
                units.append(lambda: emit_norm(1, 1))
                units.append(lambda: emit_transposes(1))
                units.append(dbg_ot)
                for mc in range(8):
                    units.append(dense_unit(mc))
            return units

        # startup: interleave weight-part and first-block x DMAs in
        # consumption order so the first projection matmuls start early
        xt0 = {}
        js0 = _ts(0, SBK)
        js1 = _ts(1, SBK)
        for xname, wsrc in (("q", wqT), ("k", wkT), ("v", wvT)):
            wr = wsrc.rearrange("(c p) m -> p c m", p=128)
            nc.sync.dma_start(out=wparts[xname], in_=wr[:, :, :])
            xt0[xname] = load_one(xname, 0, js0, fine=(xname == "q"))
        xq1 = load_one("q", 1, js1)
        nc.sync.dma_start(out=tri_sb, in_=tri_c[:, :])
        nc.sync.dma_start(out=id_sb, in_=id_c[:, :])

        def interleave(primary, secondary, sec_first=False):
            # emit primary units in order, spreading secondary units evenly
            # between them (all emission happens here)
            np_, ns = len(primary), len(secondary)
            si = 0
            for pi, u in enumerate(primary):
                if sec_first:
                    want = pi * ns // np_ + (1 if pi == 0 else 0)
                    while si < min(want, ns):
                        secondary[si]()
                        si += 1
                u()
                if not sec_first:
                    want = (pi + 1) * ns // np_
                    while si < want:
                        secondary[si]()
                        si += 1
            while si < ns:
                secondary[si]()
                si += 1

        PTs = {j: {0: [], 1: []} for j in range(NSB)}
        xts = {0: xt0, 1: {"q": xq1, "k": load_one("k", 1, js1),
                           "v": load_one("v", 1, js1)}}
        # dn is consumed only at PB(0): keep it off the startup critical path
        nc.sync.dma_start(
            out=dn_sb, in_=dnT.rearrange("(t p) n -> p t n", p=128))
        qk0, v0 = a_units(0, _ts(0, SBK), xts.pop(0), split_v=True)
        for u in qk0:
            u()
        # LB(0) logits fill the xv/x(1) DMA-wait gaps in block-0 V and A(1)
        lb0 = lb_units(0, _ts(0, SBK), PTs[0])
        interleave(v0, lb0[:4])
        for j in range(1, NSB):
            if j + 1 < NSB:
                xts[j + 1] = load_x(j + 1, _ts(j + 1, SBK))
            prim = a_units(j, _ts(j, SBK), xts.pop(j))
            sec = lb0[4:] if j == 1 else lb_units(j - 1, _ts(j - 1, SBK), PTs[j - 1])
            interleave(prim, sec, sec_first=True)
        if debug:
            nc.sync.dma_start(out=dQT[:, :, :], in_=QT_sb)
            nc.sync.dma_start(out=dKT[:, :, :], in_=KT_sb)
            nc.sync.dma_start(out=dV[:, :, :, :], in_=V_sb)
        # PB(0) interleaved with LB(3); later PBs run straight
        interleave(pb_units(0, _ts(0, SBK), PTs[0]),
                   lb_units(NSB - 1, _ts(NSB - 1, SBK), PTs[NSB - 1]),
                   sec_first=True)
        for j in range(1, NSB):
            for u in pb_units(j, _ts(j, SBK), PTs[j]):
                u()

    nc.finalize()
    return nc


_CACHE = {}


def _get_nc(causal=True, with_bq=False, with_bk=False, with_bv=False):
    key = (causal, with_bq, with_bk, with_bv)
    if key not in _CACHE:
        assert causal and not (with_bq or with_bk or with_bv)
        _CACHE[key] = build()
    return _CACHE[key]


def _numpy_fallback(query, key_, value, mask, wq_w, wq_b, wk_w, wk_b, wv_w,
                    wv_b, dense_w, dense_b):
    out = np.empty((B, S, D), np.float32)
    m4 = np.asarray(mask, np.float32).reshape(-1, S, S)
    for b in range(B):
        q = (query[b] @ wq_w.T + wq_b).reshape(S, H, DH).transpose(1, 0, 2)
        k = (key_[b] @ wk_w.T + wk_b).reshape(S, H, DH).transpose(1, 0, 2)
        v = (value[b] @ wv_w.T + wv_b).reshape(S, H, DH).transpose(1, 0, 2)
        mb = m4[min(b, m4.shape[0] - 1)]
        o = np.empty((H, S, DH), np.float32)
        for h in range(H):
            lg = (q[h] @ k[h].T) / np.sqrt(np.float32(DH)) + mb * np.float32(-1e9)
            lg -= lg.max(-1, keepdims=True)
            p = np.exp(lg)
            p /= p.sum(-1, keepdims=True)
            o[h] = p @ v[h]
        out[b] = o.transpose(1, 0, 2).reshape(S, D) @ dense_w.T + dense_b
    return out


def _prep_in_maps(query, key_, value, wq_w, wk_w, wv_w, dense_w):
    xT = {}
    for b in range(B):
        xT[b] = (
            np.ascontiguousarray(query[b].T).astype(np.float16),
            np.ascontiguousarray(key_[b].T).astype(np.float16),
            np.ascontiguousarray(value[b].T).astype(np.float16),
        )
    in_maps = []
    for c in range(NCORES):
        b, g = divmod(c, 4)
        sl = _ts(g, DLOC)
        in_maps.append({
            "xqT": xT[b][0], "xkT": xT[b][1], "xvT": xT[b][2],
            "wqT": np.ascontiguousarray(wq_w[sl].T).astype(np.float16),
            "wkT": np.ascontiguousarray(wk_w[sl].T).astype(np.float16),
            "wvT": np.ascontiguousarray(wv_w[sl].T).astype(np.float16),
            "dnT": np.ascontiguousarray(dense_w[:, sl].T).astype(np.float16),
        })
    return in_maps


def kernel(query, key_, value, mask, wq_w, wq_b, wk_w, wk_b, wv_w, wv_b,
           dense_w, dense_b, _profile_kw=None):
    query = np.asarray(query, np.float32)
    key_ = np.asarray(key_, np.float32)
    value = np.asarray(value, np.float32)
    mask2d = np.asarray(mask, np.float32).reshape(S, S)
    wq_w = np.asarray(wq_w, np.float32)
    wk_w = np.asarray(wk_w, np.float32)
    wv_w = np.asarray(wv_w, np.float32)
    dense_w = np.asarray(dense_w, np.float32)
    dense_b = np.asarray(dense_b, np.float32)

    causal = bool(np.array_equal(mask2d, np.triu(np.ones((S, S), np.float32), k=1)))
    if not causal or np.any(wq_b) or np.any(wk_b) or np.any(wv_b):
        out = _numpy_fallback(query, key_, value, mask, wq_w, wq_b, wk_w,
                              wk_b, wv_w, wv_b, dense_w, dense_b)
        return (out, None) if _profile_kw else out

    in_maps = _prep_in_maps(query, key_, value, wq_w, wk_w, wv_w, dense_w)
    nc = _get_nc(True, False, False, False)
    res = run_bass_kernel_spmd(nc, in_maps, core_ids=list(range(NCORES)),
                               **(_profile_kw or {}))

    out = np.empty((B, S, D), np.float32)
    for b in range(B):
        acc = res.results[4 * b]["outT"].astype(np.float32)
        for g in range(1, 4):
            acc = acc + res.results[4 * b + g]["outT"].astype(np.float32)
        out[b] = acc.T + dense_b[None, :]
    if _profile_kw:
        return out, res
    return out


# revision 75
# speedup vs baseline: 1.0384x; 1.0384x over previous
"""Multi-head attention (B=2, S=2048, D=1024, H=16) on 8 TRN2 NeuronCores.

Sharding: batch x head-group. Core c handles batch c//4 and heads
[4*(c%4), 4*(c%4)+4). Each core computes its heads' Q/K/V projections
(column-parallel), causal attention, and a row-parallel partial of the
output projection. The host sums the 4 partials per batch and adds dense_b.

All matmul operands are fp16 (fp32 PSUM accumulation); matmul cost is
streamed-rows-bound, so every matmul carries a full 128x128 payload:
  QT/KT [128, 2, S] fp16: partition = head-pair-local feature (2 heads x 64),
    second dim = head pair (pc). V [128p=seq, chunk, head, 65] fp16 with a
    ones column (col 64) that accumulates the softmax denominator.
  logits (phase LB): L[:, i, off:] = KT_chunk.T @ QT_block (2 heads
    row-packed via tile_position, columns below the causal diagonal
    skipped); PT = exp(0.125*L) on ScalarE (a quarter of the full chunks
    use a Schraudolph bit-trick exp on DVE instead -- ScalarE is the
    phase-B bottleneck); diagonal 128-block masked multiplicatively with
    a 0/1 lower-tri pattern on the Pool engine.
  PV (phase PB): natural orientation per head i, q-sub qs (128 wide,
    qs >= chunk diagonal): O[i][:, qs, 0:65] += PT[:, i, qs-cols].T @ V_aug
    -- K=128, M=128, only 65 streamed rows per chunk. Each O bank hosts
    strictly sequential accumulation groups (interleaved groups within one
    PSUM bank corrupt each other).
  normalize: recip = 1/O[:, :, 64] (DVE reciprocal); O8 = O * recip fused
    into the PSUM evac; OT = PE-transpose(O8) per q-sub into a shared PSUM
    tile; one DVE copy to OT_sb [128, 2, 4, 128] fp16.
  dense: outT[mc*128:+128, q-block] = sum_t dnT[:, t, mc].T @ OT_sb[:, t],
    evacuated on DVE (Act for the drain tail) and DMA'd out in 4-wide
    batches.

Emission is unit-interleaved so the in-order engines stay fed:
A0 | A1(x)LB0 | A2(x)LB1 | A3(x)LB2 | PB0(x)LB3 | PB1 | PB2 | PB3
(phase A is PE-bound while Act idles; LB is Act-bound while PE idles;
PB is PE-heavy). PT tiles are double-buffered per (pc, kc) tag so an
LB(j+1) write WARs against PB(j-1)'s long-done reads instead of
serializing the pipeline on PB(j)'s future reads.
"""

import numpy as np
from contextlib import ExitStack

import concourse.tile as tile
from concourse import bacc, mybir
from concourse.bass_utils import run_bass_kernel_spmd

F32 = mybir.dt.float32
F16 = mybir.dt.float16
I32 = mybir.dt.int32
AF = mybir.ActivationFunctionType
ADD = mybir.AluOpType.add
MULT = mybir.AluOpType.mult

B, S, D, H = 2, 2048, 1024, 16
NCORES = 8
HL = 4            # heads per core
DH = D // H       # 64
DLOC = HL * DH    # 256 local feature dims
SBK = 512         # seq block (q)
NSB = S // SBK    # 4
KCH = 128         # k chunk
NCH = S // KCH    # 16
# Schraudolph exp constants (0.125 softmax scale folded in): exp(0.125*x)
# ~= bitcast_f32(int32(x*EA + EB)); ~3% max relative error
EA = float(2 ** 23 / np.log(2) * 0.125)
EB = float(127 * 2 ** 23 - 0.043677448 * 2 ** 23)


def _ts(i, n):
    return slice(i * n, (i + 1) * n)


def build(debug=False):
    nc = bacc.Bacc(None, target_bir_lowering=False)

    xqT = nc.dram_tensor("xqT", [D, S], F16, kind="ExternalInput")
    xkT = nc.dram_tensor("xkT", [D, S], F16, kind="ExternalInput")
    xvT = nc.dram_tensor("xvT", [D, S], F16, kind="ExternalInput")
    wqT = nc.dram_tensor("wqT", [D, DLOC], F16, kind="ExternalInput")
    wkT = nc.dram_tensor("wkT", [D, DLOC], F16, kind="ExternalInput")
    wvT = nc.dram_tensor("wvT", [D, DLOC], F16, kind="ExternalInput")
    dnT = nc.dram_tensor("dnT", [DLOC, D], F16, kind="ExternalInput")
    outT = nc.dram_tensor("outT", [D, S], F16, kind="ExternalOutput")
    if debug:
        dQT = nc.dram_tensor("dQT", [128, 2, S], F16, kind="ExternalOutput")
        dKT = nc.dram_tensor("dKT", [128, 2, S], F16, kind="ExternalOutput")
        dV = nc.dram_tensor("dV", [128, NCH, HL, DH + 1], F16, kind="ExternalOutput")
        dPT = nc.dram_tensor("dPT", [128, 2, SBK], F16, kind="ExternalOutput")
        dO = nc.dram_tensor("dO", [2, 128, NSB, 128], F32, kind="ExternalOutput")
        dO8 = nc.dram_tensor("dO8", [2, 128, NSB, DH], F16, kind="ExternalOutput")
        dOT = nc.dram_tensor("dOT", [128, 2, NSB, 128], F16, kind="ExternalOutput")

    # lower-tri 0/1 pattern (allowed = k <= q within the diagonal block)
    tri_np = (np.arange(128)[:, None] <= np.arange(128)[None, :]).astype(np.float16)
    tri_c = nc.inline_tensor(tri_np, name="tri01")
    id_c = nc.inline_tensor(np.eye(128, dtype=np.float16), name="id128")

    with tile.TileContext(nc) as tc, ExitStack() as ctx:
        pers = ctx.enter_context(tc.tile_pool(name="pers", bufs=1))
        xpool = ctx.enter_context(tc.tile_pool(name="xpool", bufs=14))
        # PT pools: double-buffered tags so LB(j+1) writes WAR against
        # PB(j-1)'s (long-done) reads, not PB(j)'s future reads; kc>=12 tags
        # are only used by j=3 (single version) so they stay single-buffered
        ptpA = ctx.enter_context(tc.tile_pool(name="ptpA", bufs=2))
        ptpB = ctx.enter_context(tc.tile_pool(name="ptpB", bufs=1))
        o8p = ctx.enter_context(tc.tile_pool(name="o8p", bufs=1))
        otp = ctx.enter_context(tc.tile_pool(name="otp", bufs=2))
        evp = ctx.enter_context(tc.tile_pool(name="evp", bufs=3))
        smallp = ctx.enter_context(tc.tile_pool(name="smallp", bufs=4))
        schp = ctx.enter_context(tc.tile_pool(name="schp", bufs=2))
        mmp = ctx.enter_context(tc.tile_pool(name="mmp", bufs=2, space="PSUM"))
        lp = ctx.enter_context(tc.tile_pool(name="lp", bufs=2, space="PSUM"))
        onp = ctx.enter_context(tc.tile_pool(name="onp", bufs=1, space="PSUM"))  # 2 tags x 1 buf

        # ---------- persistent tiles ----------
        wparts = {}
        for wname in ("q", "k", "v"):
            wparts[wname] = pers.tile([128, 8, DLOC], F16, tag=f"w{wname}",
                                      name=f"w_{wname}")
        dn_sb = pers.tile([128, 2, D], F16, tag="dn")
        tri_sb = pers.tile([128, 128], F16, tag="tri")
        id_sb = pers.tile([128, 128], F16, tag="id")

        QT_sb = pers.tile([128, 2, S], F16, tag="QT")
        KT_sb = pers.tile([128, 2, S], F16, tag="KT")
        V_sb = pers.tile([128, NCH, HL, DH + 1], F16, tag="V")
        # ones column (softmax denominator accumulates via PV matmul)
        nc.vector.memset(V_sb[:, :, :, DH:DH + 1], 1.0)

        outT_r = outT.rearrange("(c p) s -> p c s", p=128)

        def load_one(xname, j, js, fine=False):
            # returns one AP of shape [128, 512] per contraction chunk kc
            src = {"q": xqT, "k": xkT, "v": xvT}[xname]
            srcr = src.rearrange("(c p) s -> p c s", p=128)
            aps = []
            for i in range(4):
                t = xpool.tile([128, 2, SBK], F16, tag="xt",
                               name=f"x_{xname}_{j}_{i}")
                nc.sync.dma_start(out=t, in_=srcr[:, _ts(i, 2), js])
                aps.extend(t[:, c, :] for c in range(2))
            return aps

        def load_x(j, js):
            return {xname: load_one(xname, j, js) for xname in ("q", "k", "v")}

        def a_units(j, js, xt, split_v=False):
            # projection series closures; evacs on Act (idle during phase A)
            units = []

            def qk_unit(bname, dst, mc):
                def f():
                    ps = mmp.tile([128, 512], F32, tag="mm",
                                  name=f"ps_{bname}_{j}_{mc}")
                    for kc in range(8):
                        nc.tensor.matmul(
                            ps[:, :],
                            lhsT=wparts[bname][:, kc, _ts(mc, 128)],
                            rhs=xt[bname][kc],
                            start=(kc == 0), stop=(kc == 7),
                        )
                    nc.scalar.copy(dst[:, mc, js], ps)
                return f

            def v_unit(sc):
                def f():
                    ps = mmp.tile([128, 512], F32, tag="mm",
                                  name=f"ps_v_{j}_{sc}")
                    for kc in range(8):
                        nc.tensor.matmul(
                            ps[:, 0:DLOC],
                            lhsT=xt["v"][kc][:, _ts(sc, 128)],
                            rhs=wparts["v"][:, kc, :],
                            start=(kc == 0), stop=(kc == 7),
                        )
                    nc.scalar.copy(
                        V_sb[:, j * 4 + sc, :, 0:DH],
                        ps[:, 0:DLOC].rearrange("p (h d) -> p h d", h=HL),
                    )
                return f

            for bname, dst in (("q", QT_sb), ("k", KT_sb)):
                for mc in range(2):
                    units.append(qk_unit(bname, dst, mc))
            vu = [v_unit(sc) for sc in range(4)]
            if split_v:
                return units, vu
            return units + vu

        def lb_units(j, js, PTs):
            # per-(pc, kc) logits+exp closures; PTs[pc][kc] filled at emission
            nkc = (j + 1) * 4

            def unit(pc, kc):
                def f():
                    off = max(0, kc - 4 * j) * KCH  # causal column trim
                    L = lp.tile([128, 2, SBK], F32, tag="L",
                                name=f"L_{j}_{pc}_{kc}")
                    for i in range(2):
                        nc.tensor.matmul(
                            L[:, i, off:SBK],
                            lhsT=KT_sb[_ts(i, 64), pc, _ts(kc, KCH)],
                            rhs=QT_sb[_ts(i, 64), pc, j * SBK + off:(j + 1) * SBK],
                            start=True, stop=True,
                            tile_position=(64 * i, 0),
                        )
                    pool = ptpA if kc < 12 else ptpB
                    PT = pool.tile([128, 2, SBK], F16, tag=f"PT{pc}_{kc}",
                                   name=f"PT_{j}_{pc}_{kc}")
                    if kc < 4 * j and kc % 4 == 1:
                        # offload this full chunk's exp to DVE (Schraudolph
                        # bit-trick): Act is the phase-B bottleneck engine
                        T = schp.tile([128, 2, SBK], I32, tag="sch",
                                      name=f"T_{j}_{pc}_{kc}")
                        nc.vector.tensor_scalar(
                            out=T, in0=L, scalar1=EA, scalar2=EB,
                            op0=MULT, op1=ADD)
                        nc.gpsimd.tensor_copy(PT, T.bitcast(F32))
                    else:
                        nc.scalar.activation(
                            out=PT[:, :, off:SBK], in_=L[:, :, off:SBK],
                            func=AF.Exp, scale=0.125)
                    if kc >= 4 * j:
                        # mask the diagonal 128-block (0/1 lower-tri multiply)
                        # on the otherwise-idle Pool engine (SBUF-only op)
                        nc.gpsimd.tensor_tensor(
                            out=PT[:, :, off:off + KCH],
                            in0=PT[:, :, off:off + KCH],
                            in1=tri_sb[:, None, :].broadcast_to([128, 2, KCH]),
                            op=MULT,
                        )
                    if debug and j == 0 and pc == 0 and kc == 0:
                        nc.sync.dma_start(out=dPT[:, :, :], in_=PT)
                    PTs[pc].append(PT)
                return f

            return [unit(pc, kc) for pc in range(2) for kc in range(nkc)]

        def pb_units(j, js, PTs):
            # PV + normalize + transpose + dense closures (PE-heavy)
            state = {}
            Os, O8s = {}, {}

            def emit_ot():
                state["OT"] = otp.tile([128, 2, NSB, 128], F16, tag="ot",
                                       name=f"OT_{j}")

            def emit_pv(pc, i):
                O = onp.tile([128, NSB, 128], F32, tag=f"o{i}",
                             name=f"O_{j}_{pc}_{i}")
                for qs in range(NSB):
                    kmax = 4 * j + qs
                    for kc in range(kmax + 1):
                        nc.tensor.matmul(
                            O[:, qs, 0:DH + 1],
                            lhsT=PTs[pc][kc][:, i, _ts(qs, 128)],
                            rhs=V_sb[:, kc, 2 * pc + i, :],
                            start=(kc == 0), stop=(kc == kmax),
                            skip_group_check=True,
                        )
                Os[pc, i] = O

            def emit_norm(pc, i):
                O = Os[pc, i]
                rc = smallp.tile([128, NSB, 1], F32, tag="rc")
                nc.vector.reciprocal(rc, O[:, :, DH:DH + 1])
                if i == 0:
                    # both heads share one tile: [128q, qs, (i, d)] so the
                    # transpose below covers the head pair in one pass
                    O8s[pc] = o8p.tile([128, NSB, 2, DH], F16, tag=f"o8{pc}",
                                       name=f"O8_{j}_{pc}")
                nc.vector.tensor_tensor(
                    out=O8s[pc][:, :, i, :], in0=O[:, :, 0:DH],
                    in1=rc.broadcast_to([128, NSB, DH]), op=MULT,
                )
                if debug and j == 0 and pc == 0:
                    ostage = evp.tile([128, NSB, 128], F32, tag="ev",
                                      name=f"ostage_{i}")
                    nc.vector.tensor_copy(ostage, O)
                    nc.sync.dma_start(out=dO[i, :, :, :], in_=ostage)
                    nc.sync.dma_start(out=dO8[i, :, :, :], in_=O8s[pc][:, :, i, :])

            def emit_transposes(pc, pool=None, tag=None):
                # one 128-wide transpose per q-sub covers both heads: out
                # partition f = i*64 + d, exactly OT's layout
                tp = (pool or mmp).tile([128, NSB, 128], F16, tag=(tag or "mm"),
                                        name=f"tp_{j}_{pc}")
                for qs in range(NSB):
                    nc.tensor.transpose(
                        tp[:, qs, :], O8s[pc][:, qs, :, :], id_sb)
                nc.vector.tensor_copy(state["OT"][:, pc, :, :], tp)

            def dense_unit(mc):
                def f():
                    OT = state["OT"]
                    last = j == NSB - 1
                    # the last block may borrow the (now idle) logits PSUM
                    # pool for a deeper dense ring
                    pool = lp if (last and mc % 2 == 1) else mmp
                    dps = pool.tile([128, 512], F32,
                                    tag=("L" if pool is lp else "mm"),
                                    name=f"dps_{j}_{mc}")
                    for t in range(2):
                        nc.tensor.matmul(
                            dps[:, :],
                            lhsT=dn_sb[:, t, _ts(mc, 128)],
                            rhs=OT[:, t, :, :].rearrange("p a b -> p (a b)"),
                            start=(t == 0), stop=(t == 1),
                        )
                    # 4-wide staging tile; one DMA per 4 mc's (fewer DMA
                    # dispatch/sem overheads); the last block's final batch
                    # goes out as two 2-wide DMAs
                    if mc % 4 == 0:
                        state["ev"] = evp.tile([128, 4, 512], F16, tag="ev",
                                               name=f"ev_{j}_{mc // 4}")
                    ev = state["ev"]
                    if last and mc >= 6:
                        nc.vector.tensor_copy(ev[:, mc % 4, 0:256], dps[:, 0:256])
                        nc.scalar.copy(ev[:, mc % 4, 256:512], dps[:, 256:512])
                    elif last and mc % 2 == 1:
                        nc.scalar.copy(ev[:, mc % 4, :], dps)
                    else:
                        nc.vector.tensor_copy(ev[:, mc % 4, :], dps)
                    if mc % 4 == 3:
                        if last and mc == 7:
                            nc.sync.dma_start(
                                out=outT_r[:, 4:6, js], in_=ev[:, 0:2, :])
                            nc.sync.dma_start(
                                out=outT_r[:, 6:8, js], in_=ev[:, 2:4, :])
                        else:
                            nc.sync.dma_start(
                                out=outT_r[:, mc - 3:mc + 1, js], in_=ev)
                return f

            def dbg_ot():
                if debug and j == 0:
                    nc.sync.dma_start(out=dOT[:, :, :, :], in_=state["OT"])

            def dense_t0_unit(mc):
                # open a dense accumulation with only the pc0 contribution
                # (emittable as soon as OT[:, 0] exists, during pv(1, 1))
                def f():
                    OT = state["OT"]
                    pool = lp if mc >= 2 else mmp
                    dps = pool.tile([128, 512], F32,
                                    tag=("L" if pool is lp else "mm"),
                                    name=f"dps_{j}_{mc}")
                    state[f"dps{mc}"] = dps
                    nc.tensor.matmul(
                        dps[:, :], lhsT=dn_sb[:, 0, _ts(mc, 128)],
                        rhs=OT[:, 0, :, :].rearrange("p a b -> p (a b)"),
                        start=True, stop=False, skip_group_check=True,
                    )
                return f

            def dense_t1_unit(mc):
                def f():
                    OT = state["OT"]
                    dps = state[f"dps{mc}"]
                    nc.tensor.matmul(
                        dps[:, :], lhsT=dn_sb[:, 1, _ts(mc, 128)],
                        rhs=OT[:, 1, :, :].rearrange("p a b -> p (a b)"),
                        start=False, stop=True, skip_group_check=True,
                    )
                    if mc % 2 == 0:
                        state["ev"] = evp.tile([128, 2, 512], F16, tag="ev",
                                               name=f"ev_{j}_{mc // 2}")
                    ev = state["ev"]
                    if mc % 2 == 1:
                        nc.scalar.copy(ev[:, 1, :], dps)
                        nc.sync.dma_start(
                            out=outT_r[:, mc - 1:mc + 1, js], in_=ev)
                    else:
                        nc.vector.tensor_copy(ev[:, 0, :], dps)
                return f

            units = [emit_ot]
            units.append(lambda: emit_pv(0, 0))
            units.append(lambda: emit_norm(0, 0))
            units.append(lambda: emit_pv(0, 1))
            units.append(lambda: emit_norm(0, 1))
            units.append(lambda: emit_pv(1, 0))
            units.append(lambda: emit_norm(1, 0))
            units.append(lambda: emit_transposes(0))
            if False:  # endgame split: no measured gain over simple path
                # endgame: pv(1,1)'s PE time covers norm(1,1); the open pc0
                # dense halves cover the pc1 transpose/OT-evac chain
                units.append(lambda: emit_pv(1, 1))
                units.append(lambda: emit_norm(1, 1))
                for mc in range(4):
                    units.append(dense_t0_unit(mc))
                units.append(lambda: emit_transposes(1, pool=onp, tag="o0"))
                units.append(dbg_ot)
                for mc in range(4):
                    units.append(dense_t1_unit(mc))
                for mc in range(4, 8):
                    units.append(dense_unit(mc))
            else:
                units.append(lambda: emit_pv(1, 1))
                units.append(lambda: emit_norm(1, 1))
                units.append(lambda: emit_transposes(1))
                units.append(dbg_ot)
                for mc in range(8):
                    units.append(dense_unit(mc))
            return units

        # startup: interleave weight-part and first-block x DMAs in
        # consumption order so the first projection matmuls start early
        xt0 = {}
        js0 = _ts(0, SBK)
        js1 = _ts(1, SBK)
        for xname, wsrc in (("q", wqT), ("k", wkT), ("v", wvT)):
            wr = wsrc.rearrange("(c p) m -> p c m", p=128)
            nc.sync.dma_start(out=wparts[xname], in_=wr[:, :, :])
            xt0[xname] = load_one(xname, 0, js0, fine=(xname == "q"))
        xq1 = load_one("q", 1, js1)
        nc.sync.dma_start(out=tri_sb, in_=tri_c[:, :])
        nc.sync.dma_start(out=id_sb, in_=id_c[:, :])

        def interleave(primary, secondary, sec_first=False):
            # emit primary units in order, spreading secondary units evenly
            # between them (all emission happens here)
            np_, ns = len(primary), len(secondary)
            si = 0
            for pi, u in enumerate(primary):
                if sec_first:
                    want = pi * ns // np_ + (1 if pi == 0 else 0)
                    while si < min(want, ns):
                        secondary[si]()
                        si += 1
                u()
                if not sec_first:
                    want = (pi + 1) * ns // np_
                    while si < want:
                        secondary[si]()
                        si += 1
            while si < ns:
                secondary[si]()
                si += 1

        PTs = {j: {0: [], 1: []} for j in range(NSB)}
        xts = {0: xt0, 1: {"q": xq1, "k": load_one("k", 1, js1),
                           "v": load_one("v", 1, js1)}}
        # dn is consumed only at PB(0): keep it off the startup critical path
        nc.sync.dma_start(
            out=dn_sb, in_=dnT.rearrange("(t p) n -> p t n", p=128))
        qk0, v0 = a_units(0, _ts(0, SBK), xts.pop(0), split_v=True)
        for u in qk0:
            u()
        # LB(0) logits fill the xv/x(1) DMA-wait gaps in block-0 V and A(1)
        lb0 = lb_units(0, _ts(0, SBK), PTs[0])
        interleave(v0, lb0[:4])
        for j in range(1, NSB):
            if j + 1 < NSB:
                xts[j + 1] = load_x(j + 1, _ts(j + 1, SBK))
            prim = a_units(j, _ts(j, SBK), xts.pop(j))
            sec = lb0[4:] if j == 1 else lb_units(j - 1, _ts(j - 1, SBK), PTs[j - 1])
            interleave(prim, sec, sec_first=True)
        if debug:
            nc.sync.dma_start(out=dQT[:, :, :], in_=QT_sb)
            nc.sync.dma_start(out=dKT[:, :, :], in_=KT_sb)
            nc.sync.dma_start(out=dV[:, :, :, :], in_=V_sb)
        # PB(0) interleaved with LB(3); later PBs run straight
        interleave(pb_units(0, _ts(0, SBK), PTs[0]),
                   lb_units(NSB - 1, _ts(NSB - 1, SBK), PTs[NSB - 1]),
                   sec_first=True)
        for j in range(1, NSB):
            for u in pb_units(j, _ts(j, SBK), PTs[j]):
                u()

    nc.finalize()
    return nc


_CACHE = {}


def _get_nc(causal=True, with_bq=False, with_bk=False, with_bv=False):
    key = (causal, with_bq, with_bk, with_bv)
    if key not in _CACHE:
        assert causal and not (with_bq or with_bk or with_bv)
        _CACHE[key] = build()
    return _CACHE[key]


def _numpy_fallback(query, key_, value, mask, wq_w, wq_b, wk_w, wk_b, wv_w,
                    wv_b, dense_w, dense_b):
    out = np.empty((B, S, D), np.float32)
    m4 = np.asarray(mask, np.float32).reshape(-1, S, S)
    for b in range(B):
        q = (query[b] @ wq_w.T + wq_b).reshape(S, H, DH).transpose(1, 0, 2)
        k = (key_[b] @ wk_w.T + wk_b).reshape(S, H, DH).transpose(1, 0, 2)
        v = (value[b] @ wv_w.T + wv_b).reshape(S, H, DH).transpose(1, 0, 2)
        mb = m4[min(b, m4.shape[0] - 1)]
        o = np.empty((H, S, DH), np.float32)
        for h in range(H):
            lg = (q[h] @ k[h].T) / np.sqrt(np.float32(DH)) + mb * np.float32(-1e9)
            lg -= lg.max(-1, keepdims=True)
            p = np.exp(lg)
            p /= p.sum(-1, keepdims=True)
            o[h] = p @ v[h]
        out[b] = o.transpose(1, 0, 2).reshape(S, D) @ dense_w.T + dense_b
    return out


def _prep_in_maps(query, key_, value, wq_w, wk_w, wv_w, dense_w):
    xT = {}
    for b in range(B):
        xT[b] = (
            np.ascontiguousarray(query[b].T).astype(np.float16),
            np.ascontiguousarray(key_[b].T).astype(np.float16),
            np.ascontiguousarray(value[b].T).astype(np.float16),
        )
    in_maps = []
    for c in range(NCORES):
        b, g = divmod(c, 4)
        sl = _ts(g, DLOC)
        in_maps.append({
            "xqT": xT[b][0], "xkT": xT[b][1], "xvT": xT[b][2],
            "wqT": np.ascontiguousarray(wq_w[sl].T).astype(np.float16),
            "wkT": np.ascontiguousarray(wk_w[sl].T).astype(np.float16),
            "wvT": np.ascontiguousarray(wv_w[sl].T).astype(np.float16),
            "dnT": np.ascontiguousarray(dense_w[:, sl].T).astype(np.float16),
        })
    return in_maps


def kernel(query, key_, value, mask, wq_w, wq_b, wk_w, wk_b, wv_w, wv_b,
           dense_w, dense_b, _profile_kw=None):
    query = np.asarray(query, np.float32)
    key_ = np.asarray(key_, np.float32)
    value = np.asarray(value, np.float32)
    mask2d = np.asarray(mask, np.float32).reshape(S, S)
    wq_w = np.asarray(wq_w, np.float32)
    wk_w = np.asarray(wk_w, np.float32)
    wv_w = np.asarray(wv_w, np.float32)
    dense_w = np.asarray(dense_w, np.float32)
    dense_b = np.asarray(dense_b, np.float32)

    causal = bool(np.array_equal(mask2d, np.triu(np.ones((S, S), np.float32), k=1)))
    if not causal or np.any(wq_b) or np.any(wk_b) or np.any(wv_b):
        out = _numpy_fallback(query, key_, value, mask, wq_w, wq_b, wk_w,
                              wk_b, wv_w, wv_b, dense_w, dense_b)
        return (out, None) if _profile_kw else out

    in_maps = _prep_in_maps(query, key_, value, wq_w, wk_w, wv_w, dense_w)
    nc = _get_nc(True, False, False, False)
    res = run_bass_kernel_spmd(nc, in_maps, core_ids=list(range(NCORES)),
                               **(_profile_kw or {}))

    out = np.empty((B, S, D), np.float32)
    for b in range(B):
        acc = res.results[4 * b]["outT"].astype(np.float32)
        for g in range(1, 4):
            acc = acc + res.results[4 * b + g]["outT"].astype(np.float32)
        out[b] = acc.T + dense_b[None, :]
    if _profile_kw:
        return out, res
    return out


# revision 76
# speedup vs baseline: 1.0505x; 1.0116x over previous
"""Multi-head attention (B=2, S=2048, D=1024, H=16) on 8 TRN2 NeuronCores.

Sharding: batch x head-group. Core c handles batch c//4 and heads
[4*(c%4), 4*(c%4)+4). Each core computes its heads' Q/K/V projections
(column-parallel), causal attention, and a row-parallel partial of the
output projection. The host sums the 4 partials per batch and adds dense_b.

All matmul operands are fp16 (fp32 PSUM accumulation); matmul cost is
streamed-rows-bound, so every matmul carries a full 128x128 payload:
  QT/KT [128, 2, S] fp16: partition = head-pair-local feature (2 heads x 64),
    second dim = head pair (pc). V [128p=seq, chunk, head, 65] fp16 with a
    ones column (col 64) that accumulates the softmax denominator.
  logits (phase LB): L[:, i, off:] = KT_chunk.T @ QT_block (2 heads
    row-packed via tile_position, columns below the causal diagonal
    skipped); PT = exp(0.125*L) on ScalarE (a quarter of the full chunks
    use a Schraudolph bit-trick exp on DVE instead -- ScalarE is the
    phase-B bottleneck); diagonal 128-block masked multiplicatively with
    a 0/1 lower-tri pattern on the Pool engine.
  PV (phase PB): natural orientation per head i, q-sub qs (128 wide,
    qs >= chunk diagonal): O[i][:, qs, 0:65] += PT[:, i, qs-cols].T @ V_aug
    -- K=128, M=128, only 65 streamed rows per chunk. Each O bank hosts
    strictly sequential accumulation groups (interleaved groups within one
    PSUM bank corrupt each other).
  normalize: recip = 1/O[:, :, 64] (DVE reciprocal); O8 = O * recip fused
    into the PSUM evac; OT = PE-transpose(O8) per q-sub into a shared PSUM
    tile; one DVE copy to OT_sb [128, 2, 4, 128] fp16.
  dense: outT[mc*128:+128, q-block] = sum_t dnT[:, t, mc].T @ OT_sb[:, t],
    evacuated on DVE (Act for the drain tail) and DMA'd out in 4-wide
    batches.

Emission is unit-interleaved so the in-order engines stay fed:
A0 | A1(x)LB0 | A2(x)LB1 | A3(x)LB2 | PB0(x)LB3 | PB1 | PB2 | PB3
(phase A is PE-bound while Act idles; LB is Act-bound while PE idles;
PB is PE-heavy). PT tiles are double-buffered per (pc, kc) tag so an
LB(j+1) write WARs against PB(j-1)'s long-done reads instead of
serializing the pipeline on PB(j)'s future reads.
"""

import numpy as np
from contextlib import ExitStack

import concourse.tile as tile
from concourse import bacc, mybir
from concourse.bass_utils import run_bass_kernel_spmd

F32 = mybir.dt.float32
F16 = mybir.dt.float16
I32 = mybir.dt.int32
AF = mybir.ActivationFunctionType
ADD = mybir.AluOpType.add
MULT = mybir.AluOpType.mult

B, S, D, H = 2, 2048, 1024, 16
NCORES = 8
HL = 4            # heads per core
DH = D // H       # 64
DLOC = HL * DH    # 256 local feature dims
SBK = 512         # seq block (q)
NSB = S // SBK    # 4
KCH = 128         # k chunk
NCH = S // KCH    # 16
# Schraudolph exp constants (0.125 softmax scale folded in): exp(0.125*x)
# ~= bitcast_f32(int32(x*EA + EB)); ~3% max relative error
EA = float(2 ** 23 / np.log(2) * 0.125)
EB = float(127 * 2 ** 23 - 0.043677448 * 2 ** 23)


def _ts(i, n):
    return slice(i * n, (i + 1) * n)


def build(debug=False):
    nc = bacc.Bacc(None, target_bir_lowering=False)

    xqT = nc.dram_tensor("xqT", [D, S], F16, kind="ExternalInput")
    xkT = nc.dram_tensor("xkT", [D, S], F16, kind="ExternalInput")
    xvT = nc.dram_tensor("xvT", [D, S], F16, kind="ExternalInput")
    wqT = nc.dram_tensor("wqT", [D, DLOC], F16, kind="ExternalInput")
    wkT = nc.dram_tensor("wkT", [D, DLOC], F16, kind="ExternalInput")
    wvT = nc.dram_tensor("wvT", [D, DLOC], F16, kind="ExternalInput")
    dnT = nc.dram_tensor("dnT", [DLOC, D], F16, kind="ExternalInput")
    outT = nc.dram_tensor("outT", [D, S], F16, kind="ExternalOutput")
    if debug:
        dQT = nc.dram_tensor("dQT", [128, 2, S], F16, kind="ExternalOutput")
        dKT = nc.dram_tensor("dKT", [128, 2, S], F16, kind="ExternalOutput")
        dV = nc.dram_tensor("dV", [128, NCH, HL, DH + 1], F16, kind="ExternalOutput")
        dPT = nc.dram_tensor("dPT", [128, 2, SBK], F16, kind="ExternalOutput")
        dO = nc.dram_tensor("dO", [2, 128, NSB, 128], F32, kind="ExternalOutput")
        dO8 = nc.dram_tensor("dO8", [2, 128, NSB, DH], F16, kind="ExternalOutput")
        dOT = nc.dram_tensor("dOT", [128, 2, NSB, 128], F16, kind="ExternalOutput")

    # lower-tri 0/1 pattern (allowed = k <= q within the diagonal block)
    tri_np = (np.arange(128)[:, None] <= np.arange(128)[None, :]).astype(np.float16)
    tri_c = nc.inline_tensor(tri_np, name="tri01")
    id_c = nc.inline_tensor(np.eye(128, dtype=np.float16), name="id128")

    with tile.TileContext(nc) as tc, ExitStack() as ctx:
        pers = ctx.enter_context(tc.tile_pool(name="pers", bufs=1))
        xpool = ctx.enter_context(tc.tile_pool(name="xpool", bufs=14))
        # PT pools: double-buffered tags so LB(j+1) writes WAR against
        # PB(j-1)'s (long-done) reads, not PB(j)'s future reads; kc>=12 tags
        # are only used by j=3 (single version) so they stay single-buffered
        ptpA = ctx.enter_context(tc.tile_pool(name="ptpA", bufs=2))
        ptpB = ctx.enter_context(tc.tile_pool(name="ptpB", bufs=1))
        o8p = ctx.enter_context(tc.tile_pool(name="o8p", bufs=1))
        otp = ctx.enter_context(tc.tile_pool(name="otp", bufs=2))
        evp = ctx.enter_context(tc.tile_pool(name="evp", bufs=3))
        smallp = ctx.enter_context(tc.tile_pool(name="smallp", bufs=4))
        schp = ctx.enter_context(tc.tile_pool(name="schp", bufs=2))
        mmp = ctx.enter_context(tc.tile_pool(name="mmp", bufs=2, space="PSUM"))
        lp = ctx.enter_context(tc.tile_pool(name="lp", bufs=2, space="PSUM"))
        onp = ctx.enter_context(tc.tile_pool(name="onp", bufs=1, space="PSUM"))  # 2 tags x 1 buf

        # ---------- persistent tiles ----------
        wparts = {}
        for wname in ("q", "k", "v"):
            wparts[wname] = pers.tile([128, 8, DLOC], F16, tag=f"w{wname}",
                                      name=f"w_{wname}")
        dn_sb = pers.tile([128, 2, D], F16, tag="dn")
        tri_sb = pers.tile([128, 128], F16, tag="tri")
        id_sb = pers.tile([128, 128], F16, tag="id")

        QT_sb = pers.tile([128, 2, S], F16, tag="QT")
        KT_sb = pers.tile([128, 2, S], F16, tag="KT")
        V_sb = pers.tile([128, NCH, HL, DH + 1], F16, tag="V")
        # ones column (softmax denominator accumulates via PV matmul)
        nc.vector.memset(V_sb[:, :, :, DH:DH + 1], 1.0)

        outT_r = outT.rearrange("(c p) s -> p c s", p=128)

        def load_one(xname, j, js, fine=False):
            # returns one AP of shape [128, 512] per contraction chunk kc
            src = {"q": xqT, "k": xkT, "v": xvT}[xname]
            srcr = src.rearrange("(c p) s -> p c s", p=128)
            aps = []
            for i in range(4):
                t = xpool.tile([128, 2, SBK], F16, tag="xt",
                               name=f"x_{xname}_{j}_{i}")
                nc.sync.dma_start(out=t, in_=srcr[:, _ts(i, 2), js])
                aps.extend(t[:, c, :] for c in range(2))
            return aps

        def load_x(j, js):
            return {xname: load_one(xname, j, js) for xname in ("q", "k", "v")}

        def a_units(j, js, xt, split_v=False):
            # projection series closures; evacs on Act (idle during phase A)
            units = []

            def qk_unit(bname, dst, mc):
                def f():
                    ps = mmp.tile([128, 512], F32, tag="mm",
                                  name=f"ps_{bname}_{j}_{mc}")
                    for kc in range(8):
                        nc.tensor.matmul(
                            ps[:, :],
                            lhsT=wparts[bname][:, kc, _ts(mc, 128)],
                            rhs=xt[bname][kc],
                            start=(kc == 0), stop=(kc == 7),
                        )
                    nc.scalar.copy(dst[:, mc, js], ps)
                return f

            def v_unit(sc):
                def f():
                    ps = mmp.tile([128, 512], F32, tag="mm",
                                  name=f"ps_v_{j}_{sc}")
                    for kc in range(8):
                        nc.tensor.matmul(
                            ps[:, 0:DLOC],
                            lhsT=xt["v"][kc][:, _ts(sc, 128)],
                            rhs=wparts["v"][:, kc, :],
                            start=(kc == 0), stop=(kc == 7),
                        )
                    nc.scalar.copy(
                        V_sb[:, j * 4 + sc, :, 0:DH],
                        ps[:, 0:DLOC].rearrange("p (h d) -> p h d", h=HL),
                    )
                return f

            for bname, dst in (("q", QT_sb), ("k", KT_sb)):
                for mc in range(2):
                    units.append(qk_unit(bname, dst, mc))
            vu = [v_unit(sc) for sc in range(4)]
            if split_v:
                return units, vu
            return units + vu

        def lb_units(j, js, PTs):
            # per-(pc, kc) logits+exp closures; PTs[pc][kc] filled at emission
            nkc = (j + 1) * 4

            def unit(pc, kc):
                def f():
                    off = max(0, kc - 4 * j) * KCH  # causal column trim
                    L = lp.tile([128, 2, SBK], F32, tag="L",
                                name=f"L_{j}_{pc}_{kc}")
                    for i in range(2):
                        nc.tensor.matmul(
                            L[:, i, off:SBK],
                            lhsT=KT_sb[_ts(i, 64), pc, _ts(kc, KCH)],
                            rhs=QT_sb[_ts(i, 64), pc, j * SBK + off:(j + 1) * SBK],
                            start=True, stop=True,
                            tile_position=(64 * i, 0),
                        )
                    pool = ptpA if kc < 12 else ptpB
                    PT = pool.tile([128, 2, SBK], F16, tag=f"PT{pc}_{kc}",
                                   name=f"PT_{j}_{pc}_{kc}")
                    if kc < 4 * j and kc % 4 == 1:
                        # offload this full chunk's exp to DVE (Schraudolph
                        # bit-trick): Act is the phase-B bottleneck engine
                        T = schp.tile([128, 2, SBK], I32, tag="sch",
                                      name=f"T_{j}_{pc}_{kc}")
                        nc.vector.tensor_scalar(
                            out=T, in0=L, scalar1=EA, scalar2=EB,
                            op0=MULT, op1=ADD)
                        nc.gpsimd.tensor_copy(PT, T.bitcast(F32))
                    else:
                        nc.scalar.activation(
                            out=PT[:, :, off:SBK], in_=L[:, :, off:SBK],
                            func=AF.Exp, scale=0.125)
                    if kc >= 4 * j:
                        # mask the diagonal 128-block (0/1 lower-tri multiply)
                        # on the otherwise-idle Pool engine (SBUF-only op)
                        nc.gpsimd.tensor_tensor(
                            out=PT[:, :, off:off + KCH],
                            in0=PT[:, :, off:off + KCH],
                            in1=tri_sb[:, None, :].broadcast_to([128, 2, KCH]),
                            op=MULT,
                        )
                    if debug and j == 0 and pc == 0 and kc == 0:
                        nc.sync.dma_start(out=dPT[:, :, :], in_=PT)
                    PTs[pc].append(PT)
                return f

            return [unit(pc, kc) for pc in range(2) for kc in range(nkc)]

        def pb_units(j, js, PTs):
            # PV + normalize + transpose + dense closures (PE-heavy)
            state = {}
            Os, O8s = {}, {}

            def emit_ot():
                state["OT"] = otp.tile([128, 2, NSB, 128], F16, tag="ot",
                                       name=f"OT_{j}")

            def emit_pv(pc, i):
                O = onp.tile([128, NSB, 128], F32, tag=f"o{i}",
                             name=f"O_{j}_{pc}_{i}")
                for qs in range(NSB):
                    kmax = 4 * j + qs
                    for kc in range(kmax + 1):
                        nc.tensor.matmul(
                            O[:, qs, 0:DH + 1],
                            lhsT=PTs[pc][kc][:, i, _ts(qs, 128)],
                            rhs=V_sb[:, kc, 2 * pc + i, :],
                            start=(kc == 0), stop=(kc == kmax),
                            skip_group_check=True,
                        )
                Os[pc, i] = O

            def emit_norm(pc, i):
                O = Os[pc, i]
                rc = smallp.tile([128, NSB, 1], F32, tag="rc")
                nc.vector.reciprocal(rc, O[:, :, DH:DH + 1])
                if i == 0:
                    # both heads share one tile: [128q, qs, (i, d)] so the
                    # transpose below covers the head pair in one pass
                    O8s[pc] = o8p.tile([128, NSB, 2, DH], F16, tag=f"o8{pc}",
                                       name=f"O8_{j}_{pc}")
                nc.vector.tensor_tensor(
                    out=O8s[pc][:, :, i, :], in0=O[:, :, 0:DH],
                    in1=rc.broadcast_to([128, NSB, DH]), op=MULT,
                )
                if debug and j == 0 and pc == 0:
                    ostage = evp.tile([128, NSB, 128], F32, tag="ev",
                                      name=f"ostage_{i}")
                    nc.vector.tensor_copy(ostage, O)
                    nc.sync.dma_start(out=dO[i, :, :, :], in_=ostage)
                    nc.sync.dma_start(out=dO8[i, :, :, :], in_=O8s[pc][:, :, i, :])

            def emit_transposes(pc, pool=None, tag=None):
                # one 128-wide transpose per q-sub covers both heads: out
                # partition f = i*64 + d, exactly OT's layout
                tp = (pool or mmp).tile([128, NSB, 128], F16, tag=(tag or "mm"),
                                        name=f"tp_{j}_{pc}")
                for qs in range(NSB):
                    nc.tensor.transpose(
                        tp[:, qs, :], O8s[pc][:, qs, :, :], id_sb)
                nc.vector.tensor_copy(state["OT"][:, pc, :, :], tp)

            def dense_unit(mc):
                def f():
                    OT = state["OT"]
                    last = j == NSB - 1
                    # the last block may borrow the (now idle) logits PSUM
                    # pool for a deeper dense ring
                    pool = lp if (last and mc % 2 == 1) else mmp
                    dps = pool.tile([128, 512], F32,
                                    tag=("L" if pool is lp else "mm"),
                                    name=f"dps_{j}_{mc}")
                    for t in range(2):
                        nc.tensor.matmul(
                            dps[:, :],
                            lhsT=dn_sb[:, t, _ts(mc, 128)],
                            rhs=OT[:, t, :, :].rearrange("p a b -> p (a b)"),
                            start=(t == 0), stop=(t == 1),
                        )
                    # 4-wide staging tile; one DMA per 4 mc's (fewer DMA
                    # dispatch/sem overheads); the last block's final batch
                    # goes out as two 2-wide DMAs
                    if mc % 4 == 0:
                        state["ev"] = evp.tile([128, 4, 512], F16, tag="ev",
                                               name=f"ev_{j}_{mc // 4}")
                    ev = state["ev"]
                    if last and mc >= 6:
                        nc.vector.tensor_copy(ev[:, mc % 4, 0:256], dps[:, 0:256])
                        nc.scalar.copy(ev[:, mc % 4, 256:512], dps[:, 256:512])
                    elif last and mc % 2 == 1:
                        nc.scalar.copy(ev[:, mc % 4, :], dps)
                    else:
                        nc.vector.tensor_copy(ev[:, mc % 4, :], dps)
                    if mc % 4 == 3:
                        if last and mc == 7:
                            nc.sync.dma_start(
                                out=outT_r[:, 4:6, js], in_=ev[:, 0:2, :])
                            nc.sync.dma_start(
                                out=outT_r[:, 6:8, js], in_=ev[:, 2:4, :])
                        else:
                            nc.sync.dma_start(
                                out=outT_r[:, mc - 3:mc + 1, js], in_=ev)
                return f

            def dbg_ot():
                if debug and j == 0:
                    nc.sync.dma_start(out=dOT[:, :, :, :], in_=state["OT"])

            def dense_t0_unit(mc):
                # open a dense accumulation with only the pc0 contribution
                # (emittable as soon as OT[:, 0] exists, during pv(1, 1))
                def f():
                    OT = state["OT"]
                    pool = lp if mc >= 2 else mmp
                    dps = pool.tile([128, 512], F32,
                                    tag=("L" if pool is lp else "mm"),
                                    name=f"dps_{j}_{mc}")
                    state[f"dps{mc}"] = dps
                    nc.tensor.matmul(
                        dps[:, :], lhsT=dn_sb[:, 0, _ts(mc, 128)],
                        rhs=OT[:, 0, :, :].rearrange("p a b -> p (a b)"),
                        start=True, stop=False, skip_group_check=True,
                    )
                return f

            def dense_t1_unit(mc):
                def f():
                    OT = state["OT"]
                    dps = state[f"dps{mc}"]
                    nc.tensor.matmul(
                        dps[:, :], lhsT=dn_sb[:, 1, _ts(mc, 128)],
                        rhs=OT[:, 1, :, :].rearrange("p a b -> p (a b)"),
                        start=False, stop=True, skip_group_check=True,
                    )
                    if mc % 2 == 0:
                        state["ev"] = evp.tile([128, 2, 512], F16, tag="ev",
                                               name=f"ev_{j}_{mc // 2}")
                    ev = state["ev"]
                    if mc % 2 == 1:
                        nc.scalar.copy(ev[:, 1, :], dps)
                        nc.sync.dma_start(
                            out=outT_r[:, mc - 1:mc + 1, js], in_=ev)
                    else:
                        nc.vector.tensor_copy(ev[:, 0, :], dps)
                return f

            units = [emit_ot]
            units.append(lambda: emit_pv(0, 0))
            units.append(lambda: emit_norm(0, 0))
            units.append(lambda: emit_pv(0, 1))
            units.append(lambda: emit_norm(0, 1))
            units.append(lambda: emit_pv(1, 0))
            units.append(lambda: emit_norm(1, 0))
            units.append(lambda: emit_transposes(0))
            if False:  # endgame split: no measured gain over simple path
                # endgame: pv(1,1)'s PE time covers norm(1,1); the open pc0
                # dense halves cover the pc1 transpose/OT-evac chain
                units.append(lambda: emit_pv(1, 1))
                units.append(lambda: emit_norm(1, 1))
                for mc in range(4):
                    units.append(dense_t0_unit(mc))
                units.append(lambda: emit_transposes(1, pool=onp, tag="o0"))
                units.append(dbg_ot)
                for mc in range(4):
                    units.append(dense_t1_unit(mc))
                for mc in range(4, 8):
                    units.append(dense_unit(mc))
            else:
                units.append(lambda: emit_pv(1, 1))
                units.append(lambda: emit_norm(1, 1))
                units.append(lambda: emit_transposes(1))
                units.append(dbg_ot)
                for mc in range(8):
                    units.append(dense_unit(mc))
            return units

        # startup: interleave weight-part and first-block x DMAs in
        # consumption order so the first projection matmuls start early
        xt0 = {}
        js0 = _ts(0, SBK)
        js1 = _ts(1, SBK)
        for xname, wsrc in (("q", wqT), ("k", wkT), ("v", wvT)):
            wr = wsrc.rearrange("(c p) m -> p c m", p=128)
            # two half-weight DMAs: first matmul starts ~700ns earlier and
            # 728ns transfers still exceed the 565ns dispatch rate
            nc.sync.dma_start(out=wparts[xname][:, 0:4, :], in_=wr[:, 0:4, :])
            nc.sync.dma_start(out=wparts[xname][:, 4:8, :], in_=wr[:, 4:8, :])
            xt0[xname] = load_one(xname, 0, js0, fine=(xname == "q"))
        xq1 = load_one("q", 1, js1)
        nc.sync.dma_start(out=tri_sb, in_=tri_c[:, :])
        nc.sync.dma_start(out=id_sb, in_=id_c[:, :])

        def interleave(primary, secondary, sec_first=False):
            # emit primary units in order, spreading secondary units evenly
            # between them (all emission happens here)
            np_, ns = len(primary), len(secondary)
            si = 0
            for pi, u in enumerate(primary):
                if sec_first:
                    want = pi * ns // np_ + (1 if pi == 0 else 0)
                    while si < min(want, ns):
                        secondary[si]()
                        si += 1
                u()
                if not sec_first:
                    want = (pi + 1) * ns // np_
                    while si < want:
                        secondary[si]()
                        si += 1
            while si < ns:
                secondary[si]()
                si += 1

        PTs = {j: {0: [], 1: []} for j in range(NSB)}
        xts = {0: xt0, 1: {"q": xq1, "k": load_one("k", 1, js1),
                           "v": load_one("v", 1, js1)}}
        # dn is consumed only at PB(0): keep it off the startup critical path
        nc.sync.dma_start(
            out=dn_sb, in_=dnT.rearrange("(t p) n -> p t n", p=128))
        qk0, v0 = a_units(0, _ts(0, SBK), xts.pop(0), split_v=True)
        for u in qk0:
            u()
        # LB(0) logits fill the xv/x(1) DMA-wait gaps in block-0 V and A(1)
        lb0 = lb_units(0, _ts(0, SBK), PTs[0])
        interleave(v0, lb0[:4])
        for j in range(1, NSB):
            if j + 1 < NSB:
                xts[j + 1] = load_x(j + 1, _ts(j + 1, SBK))
            prim = a_units(j, _ts(j, SBK), xts.pop(j))
            sec = lb0[4:] if j == 1 else lb_units(j - 1, _ts(j - 1, SBK), PTs[j - 1])
            interleave(prim, sec, sec_first=True)
        if debug:
            nc.sync.dma_start(out=dQT[:, :, :], in_=QT_sb)
            nc.sync.dma_start(out=dKT[:, :, :], in_=KT_sb)
            nc.sync.dma_start(out=dV[:, :, :, :], in_=V_sb)
        # PB(0) interleaved with LB(3); later PBs run straight
        interleave(pb_units(0, _ts(0, SBK), PTs[0]),
                   lb_units(NSB - 1, _ts(NSB - 1, SBK), PTs[NSB - 1]),
                   sec_first=True)
        for j in range(1, NSB):
            for u in pb_units(j, _ts(j, SBK), PTs[j]):
                u()

    nc.finalize()
    return nc


_CACHE = {}


def _get_nc(causal=True, with_bq=False, with_bk=False, with_bv=False):
    key = (causal, with_bq, with_bk, with_bv)
    if key not in _CACHE:
        assert causal and not (with_bq or with_bk or with_bv)
        _CACHE[key] = build()
    return _CACHE[key]


def _numpy_fallback(query, key_, value, mask, wq_w, wq_b, wk_w, wk_b, wv_w,
                    wv_b, dense_w, dense_b):
    out = np.empty((B, S, D), np.float32)
    m4 = np.asarray(mask, np.float32).reshape(-1, S, S)
    for b in range(B):
        q = (query[b] @ wq_w.T + wq_b).reshape(S, H, DH).transpose(1, 0, 2)
        k = (key_[b] @ wk_w.T + wk_b).reshape(S, H, DH).transpose(1, 0, 2)
        v = (value[b] @ wv_w.T + wv_b).reshape(S, H, DH).transpose(1, 0, 2)
        mb = m4[min(b, m4.shape[0] - 1)]
        o = np.empty((H, S, DH), np.float32)
        for h in range(H):
            lg = (q[h] @ k[h].T) / np.sqrt(np.float32(DH)) + mb * np.float32(-1e9)
            lg -= lg.max(-1, keepdims=True)
            p = np.exp(lg)
            p /= p.sum(-1, keepdims=True)
            o[h] = p @ v[h]
        out[b] = o.transpose(1, 0, 2).reshape(S, D) @ dense_w.T + dense_b
    return out


def _prep_in_maps(query, key_, value, wq_w, wk_w, wv_w, dense_w):
    xT = {}
    for b in range(B):
        xT[b] = (
            np.ascontiguousarray(query[b].T).astype(np.float16),
            np.ascontiguousarray(key_[b].T).astype(np.float16),
            np.ascontiguousarray(value[b].T).astype(np.float16),
        )
    in_maps = []
    for c in range(NCORES):
        b, g = divmod(c, 4)
        sl = _ts(g, DLOC)
        in_maps.append({
            "xqT": xT[b][0], "xkT": xT[b][1], "xvT": xT[b][2],
            "wqT": np.ascontiguousarray(wq_w[sl].T).astype(np.float16),
            "wkT": np.ascontiguousarray(wk_w[sl].T).astype(np.float16),
            "wvT": np.ascontiguousarray(wv_w[sl].T).astype(np.float16),
            "dnT": np.ascontiguousarray(dense_w[:, sl].T).astype(np.float16),
        })
    return in_maps


def kernel(query, key_, value, mask, wq_w, wq_b, wk_w, wk_b, wv_w, wv_b,
           dense_w, dense_b, _profile_kw=None):
    query = np.asarray(query, np.float32)
    key_ = np.asarray(key_, np.float32)
    value = np.asarray(value, np.float32)
    mask2d = np.asarray(mask, np.float32).reshape(S, S)
    wq_w = np.asarray(wq_w, np.float32)
    wk_w = np.asarray(wk_w, np.float32)
    wv_w = np.asarray(wv_w, np.float32)
    dense_w = np.asarray(dense_w, np.float32)
    dense_b = np.asarray(dense_b, np.float32)

    causal = bool(np.array_equal(mask2d, np.triu(np.ones((S, S), np.float32), k=1)))
    if not causal or np.any(wq_b) or np.any(wk_b) or np.any(wv_b):
        out = _numpy_fallback(query, key_, value, mask, wq_w, wq_b, wk_w,
                              wk_b, wv_w, wv_b, dense_w, dense_b)
        return (out, None) if _profile_kw else out

    in_maps = _prep_in_maps(query, key_, value, wq_w, wk_w, wv_w, dense_w)
    nc = _get_nc(True, False, False, False)
    res = run_bass_kernel_spmd(nc, in_maps, core_ids=list(range(NCORES)),
                               **(_profile_kw or {}))

    out = np.empty((B, S, D), np.float32)
    for b in range(B):
        acc = res.results[4 * b]["outT"].astype(np.float32)
        for g in range(1, 4):
            acc = acc + res.results[4 * b + g]["outT"].astype(np.float32)
        out[b] = acc.T + dense_b[None, :]
    if _profile_kw:
        return out, res
    return out
